# revision 42
# baseline (speedup 1.0000x reference)
"""Trainium2 Bass kernel for a differentiable addressing head (NTM-style).

Computes, for each batch b:
    key   = cs @ Wk;  beta = softplus(cs@Wb+bb)+1;  gate = sigmoid(cs@Wg+bg)
    shift = softmax(cs@Ws+bs);  gamma = softplus(cs@Wgam+bgam)+1
    sim   = (key . mem[n]) / (|key||mem[n]| + eps)
    cw    = softmax(beta * sim);  g = gate*cw + (1-gate)*pw
    sh    = circular_conv(g, shift);  w = (sh+1e-8)^gamma / (sum + eps)

Sharding: data-parallel over batch across 8 cores (8 batches/core).

Heavy phase per core: one pass over memory (16 MB bf16) computing dot
products on the PE (bf16, 1 col/cycle) and row norms via fp8 DoubleRow
matmuls (0.5 cyc/col) over on-chip-squared fp8 data. The DoubleRow k-tile
split is just an AP view (two contiguous 512-col halves of a 1024-col
window), so everything stays in the natural light layout.

Startup: all small tensors ride in 2 packed DMAs ahead of the memory
stripes, and a warm-up matmul stream keeps the PE busy (and ramps its
p-state to 2.4 GHz) while the first stripe is in flight.

Self-contained: hardcodes shapes B=64, N=8192, D=128, C=256.
"""

import os
import sys

import numpy as np

for _p in ("/opt/trn_rl_repo", "/opt/pypackages"):
    if _p not in sys.path and os.path.isdir(_p):
        sys.path.insert(0, _p)

import concourse.bacc as bacc
import concourse.bass as bass
import concourse.tile as tile
from concourse import mybir
from concourse.bass_utils import run_bass_kernel_spmd

F32 = mybir.dt.float32
BF16 = mybir.dt.bfloat16
F8 = mybir.dt.float8e4        # TRN e4m3 (max 240)
AF = mybir.ActivationFunctionType
OP = mybir.AluOpType
DR = mybir.MatmulPerfMode.DoubleRow

B, N, D, C = 64, 8192, 128, 256
NCORES = 8
BL = B // NCORES          # batches per core = 8
NW = 16                   # 512-wide blocks per batch row
W = N // NW               # 512, block width (= light-tile free dim)
NST = 2                   # DMA stripes per batch
SW = N // NST             # 4096, stripe width
DRW = 2 * W               # 1024, DoubleRow norm window (2 blocks)
EPS = 1e-8
NWARM = 10                # PE warm-up matmuls

# packed-constant column offsets (fp32, 128 partitions)
C_IND = 0                 # ind (128, 8)
C_INDT = 8                # indT padded (8 rows used) 128
C_ONES = 136              # ones col
C_ONE1 = 137              # 1x1 one (row 0)
C_EPS = 138               # eps col
C_S2 = 139                # strip2 (2 planes x 256)
C_OS = C_S2 + 512         # ones_strip (63)
C_SU = C_OS + 63          # shift-up perm (128): row m <- row (m+1 wrap in batch)
C_SD = C_SU + 128         # shift-down perm (128)
CC = C_SD + 128           # 970 cols

# packed-smalls column offsets (fp32, 128 partitions)
S_CST = 0                 # csT two 128-blocks (2*8)
S_WK = 16                 # Wk two 128-blocks (2*128)
S_WC = 272                # Wcat two 128-blocks (2*6)
S_B6 = 284                # bias6 (8 rows used)
SC = S_B6 + 6             # 290 cols

# Module-level caches so repeat calls don't rebuild/recompile.
_NC = None
PROFILE = False
LAST_RESULTS = None       # BassKernelResults of the last run (for profiling)


def _const_pack():
    cp = np.zeros((128, CC), np.float32)
    for p in range(128):
        cp[p, C_IND + p // NW] = 1.0          # ind[p, b] = 1 iff p//16 == b
    # indT[b, p] = 1 iff p//16 == b  (rows 0..7)
    for b in range(BL):
        for p in range(128):
            if p // NW == b:
                cp[b, C_INDT + p] = 1.0
    cp[:, C_ONES] = 1.0
    cp[0, C_ONE1] = 1.0
    cp[:, C_EPS] = EPS
    # strip2[p, k, c] = 1 iff c == 127 + k ; sliced at offset 127 - row the
    # DoubleRow matmul lands the two k-plane column sums on rows (r, r+1).
    # (the ISA requires DoubleRow weights to be full 128 columns wide)
    cp[:, C_S2 + 0 * 256 + 127] = 1.0
    cp[:, C_S2 + 1 * 256 + 128] = 1.0
    cp[:, C_OS + 31] = 1.0    # bf16 ones strip, col 31 (norms of bf16 stripes)
    # circ-conv boundary shifts: out[m] = in[perm(m)], perm wraps inside each
    # batch's 16-row block (partition p = b*16 + t).  matmul out[m] =
    # sum_p lhsT[p, m] * in[p]  =>  lhsT[p, m] = 1 iff p == perm(m).
    for m in range(128):
        b, t = m // NW, m % NW
        cp[b * NW + (t + 1) % NW, C_SU + m] = 1.0
        cp[b * NW + (t - 1) % NW, C_SD + m] = 1.0
    return cp


def _patch_act_tables():
    """Prefer the combined natural_log_exp table set. The set chooser maps
    each activation to the FIRST set containing its func, so exp->set0 and
    ln->natural_log oscillate, costing a 1.3us table load per transition on
    the critical path. natural_log_exp_and_others covers every func this
    kernel uses (exp, ln, square, copy, identity) -> one load total."""
    import concourse.hw_specs as hw_specs

    if getattr(bacc, "_act_tables_patched", False):
        return
    orig = bacc.get_activation_tables

    def filtered(module_arch):
        t = orig(module_arch)
        pref = "natural_log_exp_and_others"
        if pref in t:
            mine = {
                AF.Exp, AF.Ln, AF.Square, AF.Copy, AF.Identity, AF.MemsetZero
            } & t[pref]
            # keep dict ORDER (ids are positional); only shrink other sets
            for k in t:
                if k != pref:
                    t[k] = t[k] - mine
        return t

    bacc.get_activation_tables = filtered
    bacc._act_tables_patched = True


def build_nc():
    # Bacc (not plain Bass): its compile() runs generate_event_semaphores,
    # which splits multi-wait sync conditions to satisfy the TRN2 limit of
    # one sync wait per instruction.
    _patch_act_tables()
    nc = bacc.Bacc()

    memT_d = nc.dram_tensor("memT", [BL, D, N], BF16, kind="ExternalInput")
    smalls_d = nc.dram_tensor("smalls", [128, SC], F32, kind="ExternalInput")
    pw_d = nc.dram_tensor("pw", [BL, N], F32, kind="ExternalInput")
    out_d = nc.dram_tensor("out", [BL, N], F32, kind="ExternalOutput")
    cpack_c = nc.inline_tensor(_const_pack(), "cpack_c")

    with tile.TileContext(nc) as tc:
        with (
            tc.tile_pool(name="const", bufs=1) as cp,
            tc.tile_pool(name="mem", bufs=7) as memp,
            tc.tile_pool(name="sq", bufs=3) as sqp,
            tc.tile_pool(name="light", bufs=1) as lp,
            tc.tile_pool(name="psmm", bufs=2, space="PSUM") as psA,
            tc.tile_pool(name="pstiny", bufs=2, space="PSUM") as psB,
        ):
            # ---- packed small DMAs first: nothing big ahead of them ----
            smalls_raw = cp.tile([128, SC], F32, name="smalls_raw")
            nc.sync.dma_start(smalls_raw[:], smalls_d[:])
            cpack_raw = cp.tile([128, CC], F32, name="cpack_raw")
            nc.sync.dma_start(cpack_raw[:], cpack_c[:])

            # ---- GPSIMD staging (single producer proc for matmul inputs;
            # self-loading fp32 matmuls support only ONE sync wait, so all
            # const reads must come from one engine whose newest tick the PE
            # absorbs once). warm is staged FIRST so the PE can start early;
            # smalls_g is staged LAST so one wait covers every gpsimd tick.
            warm = cp.tile([128, 288], BF16, name="warm")
            nc.gpsimd.tensor_copy(warm[:], smalls_raw[:, 0:288])
            cpack = cp.tile([128, CC], F32, name="cpack")
            nc.gpsimd.tensor_copy(cpack[:], cpack_raw[:])
            ones_strip = cp.tile([128, 63], BF16, name="ones_strip")
            nc.gpsimd.tensor_copy(ones_strip[:], cpack_raw[:, C_OS:C_OS + 63])
            shifts_bf = cp.tile([128, 256], BF16, name="shifts_bf")
            nc.gpsimd.tensor_copy(shifts_bf[:], cpack_raw[:, C_SU:C_SU + 256])
            smalls = cp.tile([128, SC], F32, name="smalls")
            nc.gpsimd.tensor_copy(smalls[:], smalls_raw[:])

            ind_sb = cpack[:, C_IND:C_IND + BL]
            indT_sb = cpack[0:BL, C_INDT:C_INDT + 128]
            ones_col = cpack[:, C_ONES:C_ONES + 1]
            one1 = cpack[0:1, C_ONE1:C_ONE1 + 1]
            eps_col = cpack[:, C_EPS:C_EPS + 1]
            csT0 = smalls[:, S_CST:S_CST + BL]
            csT1 = smalls[:, S_CST + BL:S_CST + 2 * BL]
            Wk0 = smalls[:, S_WK:S_WK + D]
            Wk1 = smalls[:, S_WK + D:S_WK + 2 * D]
            Wc0 = smalls[:, S_WC:S_WC + 6]
            Wc1 = smalls[:, S_WC + 6:S_WC + 12]
            bias6 = smalls[0:BL, S_B6:S_B6 + 6]

            # ---- PE warm-up: keeps the PE busy (and its p-state ramping
            # toward 2.4 GHz) while the first memory stripe is in flight.
            # Write-only absorber output; also absorbs the warm gpsimd tick.
            wps = psB.tile([32, 288], F32, tag="absorb", bufs=1, name="wps")
            for i in range(NWARM):
                nc.tensor.matmul(wps[:], lhsT=warm[:, 0:32], rhs=warm[:],
                                 start=True, stop=True, skip_group_check=True)

            # ---- memory stripe prefetch + pw ----
            memT_ap = memT_d[:]
            stripes = {}
            for pb, ps in ((0, 0), (0, 1), (1, 0)):
                st = memp.tile([128, SW], BF16, tag="mst", name=f"mst_{pb}_{ps}")
                nc.sync.dma_start(st[:], memT_ap[pb][:, ps * SW:(ps + 1) * SW])
                stripes[(pb, ps)] = st
            pw_raw = cp.tile([128, W], F32)
            nc.sync.dma_start(pw_raw[:], pw_d[:].rearrange("b (q f) -> (b q) f", f=W))

            # ---- projections: key_T (D, BL) and proj (BL, 6) ----
            # (the first matmul's single wait on smalls' gpsimd tick also
            # absorbs every earlier staging op into the PE's clock)
            key_ps = psB.tile([128, BL], F32, tag="tiny")
            nc.tensor.matmul(key_ps[:], lhsT=Wk0, rhs=csT0, start=True, stop=False)
            nc.tensor.matmul(key_ps[:], lhsT=Wk1, rhs=csT1, start=False, stop=True)
            proj_ps = psB.tile([BL, 6], F32, tag="tiny")
            nc.tensor.matmul(proj_ps[:], lhsT=csT0, rhs=Wc0, start=True, stop=False)
            nc.tensor.matmul(proj_ps[:], lhsT=csT1, rhs=Wc1, start=False, stop=True)

            keyT = cp.tile([128, BL], F32)
            proj = lp.tile([BL, 6], F32)
            # DVE proc: strip2 staged first, then keyT/strips; one PE
            # absorber on strips then covers every DVE tick.
            strip2 = cp.tile([128, 2, 256], F8, name="strip2")
            nc.vector.tensor_copy(
                strip2[:], cpack_raw[:, C_S2:C_S2 + 512].rearrange(
                    "p (k c) -> p k c", k=2)
            )
            nc.vector.tensor_copy(keyT[:], key_ps[:])
            nc.vector.tensor_add(proj[:], proj_ps[:], bias6)
            pw_sb = cp.tile([128, W], F32)
            nc.vector.tensor_copy(pw_sb[:], pw_raw[:])

            # zero-padded key strips: strips[:, b, 31] = key_T[:, b]
            strips = cp.tile([128, BL, 63], BF16)
            nc.vector.memset(strips[:], 0.0)
            for b in range(BL):
                nc.vector.tensor_copy(strips[:, b, 31:32], keyT[:, b:b + 1])

            # absorb the newest DVE tick (strips) into the PE clock so the
            # dot matmuls carry only their stripe-DMA wait.
            nc.tensor.matmul(wps[0:BL, 0:1], lhsT=strips[:, :, 31],
                             rhs=strips[:, 0, 31:32],
                             start=True, stop=True, skip_group_check=True)

            # |key|^2 per batch -> (BL, 1), broadcast to partitions
            kq = lp.tile([128, BL], F32)
            nc.scalar.activation(kq[:], keyT[:], AF.Square)
            kn2_ps = psB.tile([BL, 1], F32, tag="tiny")
            nc.tensor.matmul(kn2_ps[:], lhsT=kq[:], rhs=ones_col, start=True, stop=True)
            kn2 = lp.tile([BL, 1], F32)
            nc.vector.tensor_copy(kn2[:], kn2_ps[:])
            kn2F_ps = psB.tile([128, 1], F32, tag="tiny")
            nc.tensor.matmul(kn2F_ps[:], lhsT=indT_sb, rhs=kn2[:], start=True, stop=True)
            F_kn2 = lp.tile([128, 1], F32)
            nc.vector.tensor_copy(F_kn2[:], kn2F_ps[:])

            # per-batch scalars: beta, 1-gate, s0, s1, s2, gamma  (BL, 6);
            # emitted BEFORE the heavy loop so FB and t4 are ready when the
            # row-split sim/E passes fire mid-phase.
            scal = lp.tile([BL, 6], F32)
            # softplus(x) = ln(1 + exp(x)); beta = softplus + 1
            eb = lp.tile([BL, 1], F32)
            nc.scalar.activation(eb[:], proj[:, 0:1], AF.Exp)
            sp_b = lp.tile([BL, 1], F32)
            nc.scalar.activation(sp_b[:], eb[:], AF.Ln, bias=1.0)
            nc.vector.tensor_scalar_add(scal[:, 0:1], sp_b[:], 1.0)
            # gate = sigmoid(x) = 1 / (1 + exp(-x))
            eg = lp.tile([BL, 1], F32)
            nc.scalar.activation(eg[:], proj[:, 1:2], AF.Exp, scale=-1.0)
            dg = lp.tile([BL, 1], F32)
            nc.vector.tensor_scalar_add(dg[:], eg[:], 1.0)
            gate = lp.tile([BL, 1], F32)
            nc.vector.reciprocal(gate[:], dg[:])
            nc.vector.tensor_scalar(
                scal[:, 1:2], gate[:], -1.0, 1.0, op0=OP.mult, op1=OP.add
            )
            e3 = lp.tile([BL, 3], F32)
            nc.scalar.activation(e3[:], proj[:, 2:5], AF.Exp)
            ssum = lp.tile([BL, 1], F32)
            nc.vector.reduce_sum(ssum[:], e3[:], axis=mybir.AxisListType.X)
            rssum = lp.tile([BL, 1], F32)
            nc.vector.reciprocal(rssum[:], ssum[:])
            sh3 = lp.tile([BL, 3], F32)
            nc.scalar.mul(sh3[:], e3[:], rssum[:])
            nc.vector.tensor_copy(scal[:, 2:5], sh3[:])
            # gamma = softplus(z) + 1 = ln(1 + exp(z)) + 1
            egm = lp.tile([BL, 1], F32)
            nc.scalar.activation(egm[:], proj[:, 5:6], AF.Exp)
            sp_g = lp.tile([BL, 1], F32)
            nc.scalar.activation(sp_g[:], egm[:], AF.Ln, bias=1.0)
            nc.vector.tensor_scalar_add(scal[:, 5:6], sp_g[:], 1.0)
            # broadcast to per-partition fields (128, 6)
            FB_ps = psB.tile([128, 6], F32, tag="tiny")
            nc.tensor.matmul(FB_ps[:], lhsT=indT_sb, rhs=scal[:], start=True, stop=True)
            FB = lp.tile([128, 6], F32)
            nc.vector.tensor_copy(FB[:], FB_ps[:])
            F_beta = FB[:, 0:1]
            F_g1 = FB[:, 1:2]
            F_s0 = FB[:, 2:3]
            F_s1 = FB[:, 3:4]
            F_s2 = FB[:, 4:5]
            F_gamma = FB[:, 5:6]
            t4 = lp.tile([128, W], F32)
            nc.vector.tensor_scalar_mul(t4[:], pw_sb[:], F_g1)

            # ---- heavy phase: dot[b, n] and normsq[b, n] ----
            # Per stripe: 8 bf16 dot matmuls (512-col windows, shifted-strip
            # diag trick packs block t of batch pair into PSUM row 16*(b%2)+t),
            # then the stripe is squared into a separate fp8 tile (chunks
            # rotate over DVE/ACT/GPSIMD), then the PREVIOUS stripe's 4
            # DoubleRow norm matmuls run (fp8, 2 PSUM rows per 1024-col
            # window; the k-tile split is the two 512-col halves).
            D_sb = lp.tile([128, W], F32)    # dot, light layout (p = b*16+t, f)
            NS_sb = lp.tile([128, W], F32)   # |mem|^2, light layout
            # sim/E pipeline tiles, written row-half at a time as batch-pair
            # groups complete (rows 0:64 mid-phase, 64:128 at the end) so
            # most of the serial tail chain overlaps the heavy phase.
            Lv = lp.tile([128, W], F32)
            y1 = lp.tile([128, W], F32)
            sim = lp.tile([128, W], F32)
            E = lp.tile([128, W], F32)
            rs1 = lp.tile([128, 1], F32)

            def emit_simE(hh):
                r = slice(64 * hh, 64 * hh + 64)
                # 1/(kn*mn) = exp(-0.5*ln(kn2*ns)); then E = exp(beta*sim)
                nc.scalar.activation(Lv[r, :], NS_sb[r, :], AF.Ln, scale=F_kn2[r, :])
                nc.scalar.activation(y1[r, :], Lv[r, :], AF.Exp, scale=-0.5)
                nc.vector.tensor_mul(sim[r, :], D_sb[r, :], y1[r, :])
                nc.scalar.activation(E[r, :], sim[r, :], AF.Exp,
                                     scale=F_beta[r, :], accum_out=rs1[r, :])
            # per-stripe square engine: "a"/"g" square to a separate fp8 tile
            # (DoubleRow norms, 0.21 ns/col on PE); "v" squares IN PLACE in
            # bf16 on the DVE 2x path (0.86 ns/col) and pays bf16-rate norm
            # matmuls instead -- the PE has headroom, ACT/DVE don't.
            # Group-leading stripes (0,4,8,12) must be fp8 so the group's
            # first norm matmul covers all 128 PSUM rows (start=True zeroing).
            # GPSIMD squares are slow (7.3us/stripe) -- schedule them EARLY;
            # the last batch's stripes split into half-stripe chunks spread
            # over ACT+DVE so the endgame squares finish ~2us after the DMA.
            SQ_ENG = {1: "v", 3: "v", 6: "v", 9: "v", 13: "v", 2: "g", 8: "g",
                      14: "h", 15: "h"}
            # norm emission lags the stripe by enough dots to cover the
            # square engine's latency (PE executes in order: a norm matmul
            # waiting on a late square stalls every matmul behind it)
            SQ_LAG = {"g": 4, "a": 2, "v": 2, "h": 1}
            pending = []
            dotPs = {}
            nrmPs = {}

            def emit_nrms(p):
                pb, ps, mode, sq = p
                pj = pb // 2
                rows = slice(32 * pj, 32 * pj + 32)
                half = NW * (pb % 2)
                first = (pb % 2 == 0) and (ps == 0)
                last = (pb % 2 == 1) and (ps == NST - 1)
                ndr = {"v": 0, "h": SW // DRW // 2}.get(mode, SW // DRW)
                for tl in range(ndr):      # fp8 DoubleRow windows
                    td = ps * (SW // DRW) + tl
                    o = 127 - (32 * pj + half + 2 * td)
                    nc.tensor.matmul(
                        nrmPs[pj][:, :],
                        lhsT=strip2[:, :, o:o + 128],
                        rhs=sq[:, tl * DRW:(tl + 1) * DRW].rearrange(
                            "p (k f) -> p k f", k=2),
                        perf_mode=DR,
                        start=first and (tl == 0),
                        stop=(mode != "v") and last and (tl == SW // DRW - 1)
                             and mode != "h",
                        skip_group_check=True,
                        tile_position=(0, 0),
                    )
                if mode in ("v", "h"):     # bf16 windows (squared in place)
                    st = stripes.pop((pb, ps))
                    t0 = 0 if mode == "v" else SW // W // 2
                    for tl in range(t0, SW // W):
                        t = ps * (SW // W) + tl
                        c = half + t
                        nc.tensor.matmul(
                            nrmPs[pj][rows, :],
                            lhsT=ones_strip[:, 31 - c:63 - c],
                            rhs=st[:, tl * W:(tl + 1) * W],
                            start=False,
                            stop=last and (tl == SW // W - 1),
                            skip_group_check=True,
                            tile_position=(0, 32 * pj),
                        )
                if last:
                    nc.vector.tensor_copy(NS_sb[rows, :], nrmPs[pj][rows, :])
                    if pj % 2 == 1:
                        emit_simE(pj // 2)

            for b in range(BL):
                j = b // 2
                if b % 2 == 0:
                    dotPs[j] = psA.tile([128, W], F32, tag="dotP", name=f"dotP_{j}")
                    nrmPs[j] = psA.tile([128, W], F32, tag="nrmP", bufs=3, name=f"nrmP_{j}")
                for s in range(NST):
                    if (b, s) in stripes:
                        st = stripes[(b, s)]
                    else:
                        st = memp.tile([128, SW], BF16, tag="mst", name=f"mst_{b}_{s}")
                        nc.sync.dma_start(st[:], memT_ap[b][:, s * SW:(s + 1) * SW])
                        stripes[(b, s)] = st
                    rows = slice(32 * j, 32 * j + 32)
                    for tl in range(SW // W):
                        t = s * (SW // W) + tl
                        c = NW * (b % 2) + t
                        nc.tensor.matmul(
                            dotPs[j][rows, :],
                            lhsT=strips[:, b, 31 - c:63 - c],
                            rhs=st[:, tl * W:(tl + 1) * W],
                            start=(b % 2 == 0) and (t == 0),
                            stop=(b % 2 == 1) and (t == NW - 1),
                            skip_group_check=True,
                            tile_position=(0, 32 * j),
                        )
                    if (b % 2 == 1) and (s == NST - 1):
                        # dots of this group are done; free the PSUM slot now
                        nc.vector.tensor_copy(D_sb[rows, :], dotPs[j][rows, :])
                    e = SQ_ENG.get(b * NST + s, "a")
                    sq = None
                    if e == "v":
                        nc.vector.tensor_mul(st[:], st[:], st[:])
                    elif e == "h":
                        # endgame: first half -> fp8 on ACT, second half
                        # squared in place on DVE; both engines finish ~2us
                        # after the stripe lands.
                        sq = sqp.tile([128, SW // 2], F8, tag="sq8h", bufs=2,
                                      name=f"sq_{b}_{s}")
                        nc.scalar.activation(sq[:], st[:, 0:SW // 2], AF.Square)
                        nc.vector.tensor_mul(st[:, SW // 2:SW],
                                             st[:, SW // 2:SW],
                                             st[:, SW // 2:SW])
                    else:
                        sq = sqp.tile([128, SW], F8, tag="sq8", name=f"sq_{b}_{s}")
                        if e == "a":
                            nc.scalar.activation(sq[:], st[:], AF.Square)
                        else:
                            nc.gpsimd.tensor_mul(sq[:], st[:], st[:])
                        stripes.pop((b, s))
                    pending.append((b, s, e, sq))
                    while pending:
                        idx = b * NST + s
                        h0 = pending[0]
                        hidx = h0[0] * NST + h0[1]
                        if idx - hidx >= SQ_LAG[h0[2]]:
                            emit_nrms(pending.pop(0))
                        else:
                            break
            for p in pending:
                emit_nrms(p)

            # ---- light phase tail ----
            S_ps = psB.tile([BL, 1], F32, tag="tiny")
            nc.tensor.matmul(S_ps[:], lhsT=ind_sb, rhs=rs1[:], start=True, stop=True)
            rS = lp.tile([BL, 1], F32)
            nc.vector.reciprocal(rS[:], S_ps[:])
            gs = lp.tile([BL, 1], F32)
            nc.vector.tensor_mul(gs[:], gate[:], rS[:])
            F2_ps = psB.tile([128, 1], F32, tag="tiny")
            nc.tensor.matmul(F2_ps[:], lhsT=indT_sb, rhs=gs[:], start=True, stop=True)
            F_gs = lp.tile([128, 1], F32)
            nc.vector.tensor_copy(F_gs[:], F2_ps[:])

            # gated = gs*E + (1-gate)*pw   (gs = gate/softmax_sum; t4 ready)
            Esc = lp.tile([128, 1], F32)
            nc.vector.tensor_copy(Esc[:], E[:, 0:1])  # DVE observes ACT@E
            G = lp.tile([128, W], F32)
            nc.vector.scalar_tensor_tensor(
                G[:], E[:], F_gs[:], t4[:], op0=OP.mult, op1=OP.add
            )

            # circular conv: SH = s1*G + s0*roll(G,-1) + s2*roll(G,+1)
            SH = lp.tile([128, W], F32)
            nc.vector.tensor_scalar_mul(SH[:], G[:], F_s1)
            nc.vector.scalar_tensor_tensor(
                SH[:, 0:W - 1], G[:, 1:W], F_s0, SH[:, 0:W - 1],
                op0=OP.mult, op1=OP.add,
            )
            nc.vector.scalar_tensor_tensor(
                SH[:, 1:W], G[:, 0:W - 1], F_s2, SH[:, 1:W],
                op0=OP.mult, op1=OP.add,
            )
            # boundary columns: SH[p, W-1] += s0 * G[p+1 (wrap in batch), 0]
            # and SH[p, 0] += s2 * G[p-1 (wrap), W-1].  One bf16 matmul per
            # direction against a constant in-batch shift permutation does
            # the partition shift (out[m] = sum_p perm[p, m] * G[p, col]).
            Gb = lp.tile([128, 2], BF16)
            nc.vector.tensor_copy(Gb[:, 0:1], G[:, 0:1])
            nc.vector.tensor_copy(Gb[:, 1:2], G[:, W - 1:W])
            bl_ps = psB.tile([128, 1], F32, tag="tiny")
            nc.tensor.matmul(bl_ps[:], lhsT=shifts_bf[:, 0:128], rhs=Gb[:, 0:1],
                             start=True, stop=True)
            bl = lp.tile([128, 1], F32)
            nc.vector.tensor_copy(bl[:], bl_ps[:])
            nc.vector.scalar_tensor_tensor(
                SH[:, W - 1:W], bl[:], F_s0, SH[:, W - 1:W],
                op0=OP.mult, op1=OP.add,
            )
            br_ps = psB.tile([128, 1], F32, tag="tiny")
            nc.tensor.matmul(br_ps[:], lhsT=shifts_bf[:, 128:256], rhs=Gb[:, 1:2],
                             start=True, stop=True)
            br = lp.tile([128, 1], F32)
            nc.vector.tensor_copy(br[:], br_ps[:])
            nc.vector.scalar_tensor_tensor(
                SH[:, 0:1], br[:], F_s2, SH[:, 0:1], op0=OP.mult, op1=OP.add
            )

            # sharpening: P2 = (SH + 1e-8)^gamma = exp(gamma * ln(SH + 1e-8))
            Lg = lp.tile([128, W], F32)
            nc.scalar.activation(Lg[:], SH[:], AF.Ln, bias=eps_col)
            P2 = lp.tile([128, W], F32)
            rs2 = lp.tile([128, 1], F32)
            nc.scalar.activation(P2[:], Lg[:], AF.Exp, scale=F_gamma, accum_out=rs2[:])
            S2_ps = psB.tile([BL, 1], F32, tag="tiny")
            nc.tensor.matmul(S2_ps[:], lhsT=ind_sb, rhs=rs2[:], start=True, stop=True)
            S2 = lp.tile([BL, 1], F32)
            nc.vector.tensor_scalar_add(S2[:], S2_ps[:], EPS)
            r2 = lp.tile([BL, 1], F32)
            nc.vector.reciprocal(r2[:], S2[:])
            F3_ps = psB.tile([128, 1], F32, tag="tiny")
            nc.tensor.matmul(F3_ps[:], lhsT=indT_sb, rhs=r2[:], start=True, stop=True)
            F_r2 = lp.tile([128, 1], F32)
            nc.vector.tensor_copy(F_r2[:], F3_ps[:])

            P2sc = lp.tile([128, 1], F32)
            nc.vector.tensor_copy(P2sc[:], P2[:, 0:1])  # DVE observes ACT@P2
            outsb = lp.tile([128, W], F32)
            nc.vector.tensor_scalar_mul(outsb[:], P2[:], F_r2[:])
            nc.sync.dma_start(
                out_d[:].rearrange("b (q f) -> (b q) f", f=W), outsb[:]
            )
    nc.compile()
    return nc


def _get_nc():
    global _NC
    if _NC is None:
        _NC = build_nc()
    return _NC


def _enable_profiling():
    """Install the axon NTFF profile hook; the agent image lacks
    antenv.axon_hooks, so shim it and register the ctypes-based hook."""
    import types

    import concourse.bass_utils as bu

    bu.upload_artifacts = lambda tmpdir: tmpdir  # no artifact bucket here
    try:
        from antenv.axon_hooks import get_axon_ntff_profile_hook  # noqa: F401

        return
    except ImportError:
        pass
    import antenv

    mod = types.ModuleType("antenv.axon_hooks")
    _holder = {}
    mod.set_axon_ntff_profile_hook = lambda h: _holder.__setitem__("h", h)
    mod.get_axon_ntff_profile_hook = lambda: _holder.get("h")
    sys.modules["antenv.axon_hooks"] = mod
    antenv.axon_hooks = mod
    from trn_agent_boot.trn_boot import _ntff_profile_via_ctypes

    mod.set_axon_ntff_profile_hook(
        _ntff_profile_via_ctypes("/opt/axon/libaxon_pjrt.so")
    )


def kernel(**inputs):
    global LAST_RESULTS
    mem = np.ascontiguousarray(np.asarray(inputs["memory"], dtype=np.float32))
    cs = np.ascontiguousarray(np.asarray(inputs["controller_state"], dtype=np.float32))
    pw = np.ascontiguousarray(np.asarray(inputs["previous_weights"], dtype=np.float32))
    Wk = np.asarray(inputs["Wk"], np.float32)
    Wcat = np.concatenate(
        [
            np.asarray(inputs["Wb"], np.float32),
            np.asarray(inputs["Wg"], np.float32),
            np.asarray(inputs["Ws"], np.float32),
            np.asarray(inputs["Wgam"], np.float32),
        ],
        axis=1,
    )
    brow = np.concatenate(
        [
            np.asarray(inputs["bb"], np.float32),
            np.asarray(inputs["bg"], np.float32),
            np.asarray(inputs["bs"], np.float32),
            np.asarray(inputs["bgam"], np.float32),
        ]
    )

    # shard: core c gets batches [c*BL, (c+1)*BL); memory pre-transposed to (BL, D, N)
    memT = np.ascontiguousarray(
        mem.reshape(NCORES, BL, N, D).transpose(0, 1, 3, 2)
    )
    import ml_dtypes
    memT = memT.astype(ml_dtypes.bfloat16)
    csT = cs.reshape(NCORES, BL, C).transpose(0, 2, 1)  # (cores, C, BL)
    pw_sh = pw.reshape(NCORES, BL, N)

    smalls = np.zeros((NCORES, 128, SC), np.float32)
    smalls[:, :, S_CST:S_CST + BL] = csT[:, 0:128, :]
    smalls[:, :, S_CST + BL:S_CST + 2 * BL] = csT[:, 128:256, :]
    smalls[:, :, S_WK:S_WK + D] = Wk[0:128, :]
    smalls[:, :, S_WK + D:S_WK + 2 * D] = Wk[128:256, :]
    smalls[:, :, S_WC:S_WC + 6] = Wcat[0:128, :]
    smalls[:, :, S_WC + 6:S_WC + 12] = Wcat[128:256, :]
    smalls[:, 0:BL, S_B6:S_B6 + 6] = brow[None, None, :]

    in_maps = [
        {
            "memT": memT[c],
            "smalls": np.ascontiguousarray(smalls[c]),
            "pw": np.ascontiguousarray(pw_sh[c]),
        }
        for c in range(NCORES)
    ]
    nc = _get_nc()
    if PROFILE:
        _enable_profiling()
    res = run_bass_kernel_spmd(nc, in_maps, list(range(NCORES)), trace=PROFILE)
    LAST_RESULTS = res
    out = np.concatenate([r["out"] for r in res.results], axis=0)
    return out.astype(np.float32)


# revision 45
# speedup vs baseline: 1.0774x; 1.0774x over previous
"""Trainium2 Bass kernel for a differentiable addressing head (NTM-style).

Computes, for each batch b:
    key   = cs @ Wk;  beta = softplus(cs@Wb+bb)+1;  gate = sigmoid(cs@Wg+bg)
    shift = softmax(cs@Ws+bs);  gamma = softplus(cs@Wgam+bgam)+1
    sim   = (key . mem[n]) / (|key||mem[n]| + eps)
    cw    = softmax(beta * sim);  g = gate*cw + (1-gate)*pw
    sh    = circular_conv(g, shift);  w = (sh+1e-8)^gamma / (sum + eps)

Sharding: data-parallel over batch across 8 cores (8 batches/core).

Heavy phase per core: one pass over memory (16 MB bf16) computing dot
products on the PE (bf16, 1 col/cycle) and row norms via fp8 DoubleRow
matmuls (0.5 cyc/col) over on-chip-squared fp8 data. The DoubleRow k-tile
split is just an AP view (two contiguous 512-col halves of a 1024-col
window), so everything stays in the natural light layout.

Startup: all small tensors ride in 2 packed DMAs ahead of the memory
stripes, and a warm-up matmul stream keeps the PE busy (and ramps its
p-state to 2.4 GHz) while the first stripe is in flight.

Self-contained: hardcodes shapes B=64, N=8192, D=128, C=256.
"""

import os
import sys

import numpy as np

for _p in ("/opt/trn_rl_repo", "/opt/pypackages"):
    if _p not in sys.path and os.path.isdir(_p):
        sys.path.insert(0, _p)

import concourse.bacc as bacc
import concourse.bass as bass
import concourse.tile as tile
from concourse import mybir
from concourse.bass_utils import run_bass_kernel_spmd

F32 = mybir.dt.float32
BF16 = mybir.dt.bfloat16
F8 = mybir.dt.float8e4        # TRN e4m3 (max 240)
AF = mybir.ActivationFunctionType
OP = mybir.AluOpType
DR = mybir.MatmulPerfMode.DoubleRow

B, N, D, C = 64, 8192, 128, 256
NCORES = 8
BL = B // NCORES          # batches per core = 8
NW = 16                   # 512-wide blocks per batch row
W = N // NW               # 512, block width (= light-tile free dim)
NST = 2                   # DMA stripes per batch
SW = N // NST             # 4096, stripe width
DRW = 2 * W               # 1024, DoubleRow norm window (2 blocks)
EPS = 1e-8
NWARM = 10                # PE warm-up matmuls

# packed-constant column offsets (fp32, 128 partitions)
C_IND = 0                 # ind (128, 8)
C_INDT = 8                # indT padded (8 rows used) 128
C_ONES = 136              # ones col
C_ONE1 = 137              # 1x1 one (row 0)
C_EPS = 138               # eps col
C_S2 = 139                # strip2 (2 planes x 256)
C_OS = C_S2 + 512         # ones_strip (63)
C_SU = C_OS + 63          # shift-up perm (128): row m <- row (m+1 wrap in batch)
C_SD = C_SU + 128         # shift-down perm (128)
CC = C_SD + 128           # 970 cols

# packed-smalls column offsets (fp32, 128 partitions)
S_CST = 0                 # csT two 128-blocks (2*8)
S_WK = 16                 # Wk two 128-blocks (2*128)
S_WC = 272                # Wcat two 128-blocks (2*6)
S_B6 = 284                # bias6 (8 rows used)
SC = S_B6 + 6             # 290 cols

# Module-level caches so repeat calls don't rebuild/recompile.
_NC = None
PROFILE = False
LAST_RESULTS = None       # BassKernelResults of the last run (for profiling)


def _const_pack():
    cp = np.zeros((128, CC), np.float32)
    for p in range(128):
        cp[p, C_IND + p // NW] = 1.0          # ind[p, b] = 1 iff p//16 == b
    # indT[b, p] = 1 iff p//16 == b  (rows 0..7)
    for b in range(BL):
        for p in range(128):
            if p // NW == b:
                cp[b, C_INDT + p] = 1.0
    cp[:, C_ONES] = 1.0
    cp[0, C_ONE1] = 1.0
    cp[:, C_EPS] = EPS
    # strip2[p, k, c] = 1 iff c == 127 + k ; sliced at offset 127 - row the
    # DoubleRow matmul lands the two k-plane column sums on rows (r, r+1).
    # (the ISA requires DoubleRow weights to be full 128 columns wide)
    cp[:, C_S2 + 0 * 256 + 127] = 1.0
    cp[:, C_S2 + 1 * 256 + 128] = 1.0
    cp[:, C_OS + 31] = 1.0    # bf16 ones strip, col 31 (norms of bf16 stripes)
    # circ-conv boundary shifts: out[m] = in[perm(m)], perm wraps inside each
    # batch's 16-row block (partition p = b*16 + t).  matmul out[m] =
    # sum_p lhsT[p, m] * in[p]  =>  lhsT[p, m] = 1 iff p == perm(m).
    for m in range(128):
        b, t = m // NW, m % NW
        cp[b * NW + (t + 1) % NW, C_SU + m] = 1.0
        cp[b * NW + (t - 1) % NW, C_SD + m] = 1.0
    return cp


def _patch_act_tables():
    """Prefer the combined natural_log_exp table set. The set chooser maps
    each activation to the FIRST set containing its func, so exp->set0 and
    ln->natural_log oscillate, costing a 1.3us table load per transition on
    the critical path. natural_log_exp_and_others covers every func this
    kernel uses (exp, ln, square, copy, identity) -> one load total."""
    import concourse.hw_specs as hw_specs

    if getattr(bacc, "_act_tables_patched", False):
        return
    orig = bacc.get_activation_tables

    def filtered(module_arch):
        t = orig(module_arch)
        pref = "natural_log_exp_and_others"
        if pref in t:
            mine = {
                AF.Exp, AF.Ln, AF.Square, AF.Copy, AF.Identity, AF.MemsetZero
            } & t[pref]
            # keep dict ORDER (ids are positional); only shrink other sets
            for k in t:
                if k != pref:
                    t[k] = t[k] - mine
        return t

    bacc.get_activation_tables = filtered
    bacc._act_tables_patched = True


def build_nc():
    # Bacc (not plain Bass): its compile() runs generate_event_semaphores,
    # which splits multi-wait sync conditions to satisfy the TRN2 limit of
    # one sync wait per instruction.
    _patch_act_tables()
    nc = bacc.Bacc()

    memT_d = nc.dram_tensor("memT", [BL, D, N], BF16, kind="ExternalInput")
    smalls_d = nc.dram_tensor("smalls", [128, SC], F32, kind="ExternalInput")
    pw_d = nc.dram_tensor("pw", [BL, N], F32, kind="ExternalInput")
    out_d = nc.dram_tensor("out", [BL, N], F32, kind="ExternalOutput")
    cpack_c = nc.inline_tensor(_const_pack(), "cpack_c")

    with tile.TileContext(nc) as tc:
        with (
            tc.tile_pool(name="const", bufs=1) as cp,
            tc.tile_pool(name="mem", bufs=7) as memp,
            tc.tile_pool(name="sq", bufs=3) as sqp,
            tc.tile_pool(name="light", bufs=1) as lp,
            tc.tile_pool(name="psmm", bufs=2, space="PSUM") as psA,
            tc.tile_pool(name="pstiny", bufs=2, space="PSUM") as psB,
        ):
            # ---- packed small DMAs first: nothing big ahead of them ----
            smalls_raw = cp.tile([128, SC], F32, name="smalls_raw")
            nc.sync.dma_start(smalls_raw[:], smalls_d[:])
            cpack_raw = cp.tile([128, CC], F32, name="cpack_raw")
            nc.sync.dma_start(cpack_raw[:], cpack_c[:])

            # ---- GPSIMD staging (single producer proc for matmul inputs;
            # self-loading fp32 matmuls support only ONE sync wait, so all
            # const reads must come from one engine whose newest tick the PE
            # absorbs once). warm is staged FIRST so the PE can start early;
            # smalls_g is staged LAST so one wait covers every gpsimd tick.
            warm = cp.tile([128, 288], BF16, name="warm")
            nc.gpsimd.tensor_copy(warm[:], smalls_raw[:, 0:288])
            cpack = cp.tile([128, CC], F32, name="cpack")
            nc.gpsimd.tensor_copy(cpack[:], cpack_raw[:])
            ones_strip = cp.tile([128, 63], BF16, name="ones_strip")
            nc.gpsimd.tensor_copy(ones_strip[:], cpack_raw[:, C_OS:C_OS + 63])
            shifts_bf = cp.tile([128, 256], BF16, name="shifts_bf")
            nc.gpsimd.tensor_copy(shifts_bf[:], cpack_raw[:, C_SU:C_SU + 256])
            smalls = cp.tile([128, SC], F32, name="smalls")
            nc.gpsimd.tensor_copy(smalls[:], smalls_raw[:])

            ind_sb = cpack[:, C_IND:C_IND + BL]
            indT_sb = cpack[0:BL, C_INDT:C_INDT + 128]
            ones_col = cpack[:, C_ONES:C_ONES + 1]
            one1 = cpack[0:1, C_ONE1:C_ONE1 + 1]
            eps_col = cpack[:, C_EPS:C_EPS + 1]
            csT0 = smalls[:, S_CST:S_CST + BL]
            csT1 = smalls[:, S_CST + BL:S_CST + 2 * BL]
            Wk0 = smalls[:, S_WK:S_WK + D]
            Wk1 = smalls[:, S_WK + D:S_WK + 2 * D]
            Wc0 = smalls[:, S_WC:S_WC + 6]
            Wc1 = smalls[:, S_WC + 6:S_WC + 12]
            bias6 = smalls[0:BL, S_B6:S_B6 + 6]

            # ---- PE warm-up: keeps the PE busy (and its p-state ramping
            # toward 2.4 GHz) while the first memory stripe is in flight.
            # Write-only absorber output; also absorbs the warm gpsimd tick.
            wps = psB.tile([32, 288], F32, tag="absorb", bufs=1, name="wps")
            for i in range(NWARM):
                nc.tensor.matmul(wps[:], lhsT=warm[:, 0:32], rhs=warm[:],
                                 start=True, stop=True, skip_group_check=True)

            # ---- memory stripe prefetch + pw ----
            memT_ap = memT_d[:]
            stripes = {}
            for pb, ps in ((0, 0), (0, 1), (1, 0)):
                st = memp.tile([128, SW], BF16, tag="mst", name=f"mst_{pb}_{ps}")
                nc.sync.dma_start(st[:], memT_ap[pb][:, ps * SW:(ps + 1) * SW])
                stripes[(pb, ps)] = st
            pw_raw = cp.tile([128, W], F32)
            nc.sync.dma_start(pw_raw[:], pw_d[:].rearrange("b (q f) -> (b q) f", f=W))

            # ---- projections: key_T (D, BL) and proj (BL, 6) ----
            # (the first matmul's single wait on smalls' gpsimd tick also
            # absorbs every earlier staging op into the PE's clock)
            key_ps = psB.tile([128, BL], F32, tag="tiny")
            nc.tensor.matmul(key_ps[:], lhsT=Wk0, rhs=csT0, start=True, stop=False)
            nc.tensor.matmul(key_ps[:], lhsT=Wk1, rhs=csT1, start=False, stop=True)
            proj_ps = psB.tile([BL, 6], F32, tag="tiny")
            nc.tensor.matmul(proj_ps[:], lhsT=csT0, rhs=Wc0, start=True, stop=False)
            nc.tensor.matmul(proj_ps[:], lhsT=csT1, rhs=Wc1, start=False, stop=True)

            keyT = cp.tile([128, BL], F32)
            proj = lp.tile([BL, 6], F32)
            # DVE proc: strip2 staged first, then keyT/strips; one PE
            # absorber on strips then covers every DVE tick.
            strip2 = cp.tile([128, 2, 256], F8, name="strip2")
            nc.vector.tensor_copy(
                strip2[:], cpack_raw[:, C_S2:C_S2 + 512].rearrange(
                    "p (k c) -> p k c", k=2)
            )
            nc.vector.tensor_copy(keyT[:], key_ps[:])
            nc.vector.tensor_add(proj[:], proj_ps[:], bias6)
            pw_sb = cp.tile([128, W], F32)
            nc.vector.tensor_copy(pw_sb[:], pw_raw[:])

            # zero-padded key strips: strips[:, b, 31] = key_T[:, b]
            strips = cp.tile([128, BL, 63], BF16)
            nc.vector.memset(strips[:], 0.0)
            for b in range(BL):
                nc.vector.tensor_copy(strips[:, b, 31:32], keyT[:, b:b + 1])

            # absorb the newest DVE tick (strips) into the PE clock so the
            # dot matmuls carry only their stripe-DMA wait.
            nc.tensor.matmul(wps[0:BL, 0:1], lhsT=strips[:, :, 31],
                             rhs=strips[:, 0, 31:32],
                             start=True, stop=True, skip_group_check=True)

            # |key|^2 per batch -> (BL, 1), broadcast to partitions
            kq = lp.tile([128, BL], F32)
            nc.scalar.activation(kq[:], keyT[:], AF.Square)
            kn2_ps = psB.tile([BL, 1], F32, tag="tiny")
            nc.tensor.matmul(kn2_ps[:], lhsT=kq[:], rhs=ones_col, start=True, stop=True)
            kn2 = lp.tile([BL, 1], F32)
            nc.vector.tensor_copy(kn2[:], kn2_ps[:])
            kn2F_ps = psB.tile([128, 1], F32, tag="tiny")
            nc.tensor.matmul(kn2F_ps[:], lhsT=indT_sb, rhs=kn2[:], start=True, stop=True)
            F_kn2 = lp.tile([128, 1], F32)
            nc.vector.tensor_copy(F_kn2[:], kn2F_ps[:])

            # per-batch scalars: beta, 1-gate, s0, s1, s2, gamma  (BL, 6);
            # emitted BEFORE the heavy loop so FB and t4 are ready when the
            # row-split sim/E passes fire mid-phase.
            scal = lp.tile([BL, 6], F32)
            # softplus(x) = ln(1 + exp(x)); beta = softplus + 1
            eb = lp.tile([BL, 1], F32)
            nc.scalar.activation(eb[:], proj[:, 0:1], AF.Exp)
            sp_b = lp.tile([BL, 1], F32)
            nc.scalar.activation(sp_b[:], eb[:], AF.Ln, bias=1.0)
            nc.vector.tensor_scalar_add(scal[:, 0:1], sp_b[:], 1.0)
            # gate = sigmoid(x) = 1 / (1 + exp(-x))
            eg = lp.tile([BL, 1], F32)
            nc.scalar.activation(eg[:], proj[:, 1:2], AF.Exp, scale=-1.0)
            dg = lp.tile([BL, 1], F32)
            nc.vector.tensor_scalar_add(dg[:], eg[:], 1.0)
            gate = lp.tile([BL, 1], F32)
            nc.vector.reciprocal(gate[:], dg[:])
            nc.vector.tensor_scalar(
                scal[:, 1:2], gate[:], -1.0, 1.0, op0=OP.mult, op1=OP.add
            )
            e3 = lp.tile([BL, 3], F32)
            nc.scalar.activation(e3[:], proj[:, 2:5], AF.Exp)
            ssum = lp.tile([BL, 1], F32)
            nc.vector.reduce_sum(ssum[:], e3[:], axis=mybir.AxisListType.X)
            rssum = lp.tile([BL, 1], F32)
            nc.vector.reciprocal(rssum[:], ssum[:])
            sh3 = lp.tile([BL, 3], F32)
            nc.scalar.mul(sh3[:], e3[:], rssum[:])
            nc.vector.tensor_copy(scal[:, 2:5], sh3[:])
            # gamma = softplus(z) + 1 = ln(1 + exp(z)) + 1
            egm = lp.tile([BL, 1], F32)
            nc.scalar.activation(egm[:], proj[:, 5:6], AF.Exp)
            sp_g = lp.tile([BL, 1], F32)
            nc.scalar.activation(sp_g[:], egm[:], AF.Ln, bias=1.0)
            nc.vector.tensor_scalar_add(scal[:, 5:6], sp_g[:], 1.0)
            # broadcast to per-partition fields (128, 6)
            FB_ps = psB.tile([128, 6], F32, tag="tiny")
            nc.tensor.matmul(FB_ps[:], lhsT=indT_sb, rhs=scal[:], start=True, stop=True)
            FB = lp.tile([128, 6], F32)
            nc.vector.tensor_copy(FB[:], FB_ps[:])
            F_beta = FB[:, 0:1]
            F_g1 = FB[:, 1:2]
            F_s0 = FB[:, 2:3]
            F_s1 = FB[:, 3:4]
            F_s2 = FB[:, 4:5]
            F_gamma = FB[:, 5:6]
            t4 = lp.tile([128, W], F32)
            nc.vector.tensor_scalar_mul(t4[:], pw_sb[:], F_g1)

            # ---- heavy phase: dot[b, n] and normsq[b, n] ----
            # Per stripe: 8 bf16 dot matmuls (512-col windows, shifted-strip
            # diag trick packs block t of batch pair into PSUM row 16*(b%2)+t),
            # then the stripe is squared into a separate fp8 tile (chunks
            # rotate over DVE/ACT/GPSIMD), then the PREVIOUS stripe's 4
            # DoubleRow norm matmuls run (fp8, 2 PSUM rows per 1024-col
            # window; the k-tile split is the two 512-col halves).
            D_sb = lp.tile([128, W], F32)    # dot, light layout (p = b*16+t, f)
            NS_sb = lp.tile([128, W], F32)   # |mem|^2, light layout
            # sim/E pipeline tiles, written row-half at a time as batch-pair
            # groups complete (rows 0:64 mid-phase, 64:128 at the end) so
            # most of the serial tail chain overlaps the heavy phase.
            Lv = lp.tile([128, W], F32)
            y1 = lp.tile([128, W], F32)
            sim = lp.tile([128, W], F32)
            E = lp.tile([128, W], F32)
            rs1 = lp.tile([128, 1], F32)

            def emit_simE(hh):
                r = slice(64 * hh, 64 * hh + 64)
                # 1/(kn*mn) = exp(-0.5*ln(kn2*ns)); then E = exp(beta*sim)
                nc.scalar.activation(Lv[r, :], NS_sb[r, :], AF.Ln, scale=F_kn2[r, :])
                nc.scalar.activation(y1[r, :], Lv[r, :], AF.Exp, scale=-0.5)
                nc.vector.tensor_mul(sim[r, :], D_sb[r, :], y1[r, :])
                nc.scalar.activation(E[r, :], sim[r, :], AF.Exp,
                                     scale=F_beta[r, :], accum_out=rs1[r, :])
            # per-stripe square engine: "a"/"g" square to a separate fp8 tile
            # (DoubleRow norms, 0.21 ns/col on PE); "v" squares IN PLACE in
            # bf16 on the DVE 2x path (0.86 ns/col) and pays bf16-rate norm
            # matmuls instead -- the PE has headroom, ACT/DVE don't.
            # Group-leading stripes (0,4,8,12) must be fp8 so the group's
            # first norm matmul covers all 128 PSUM rows (start=True zeroing).
            # GPSIMD squares are slow (7.3us/stripe) -- schedule them EARLY;
            # the last batch's stripes split into half-stripe chunks spread
            # over ACT+DVE so the endgame squares finish ~2us after the DMA.
            SQ_ENG = {1: "v", 5: "v", 9: "v", 2: "g", 14: "h", 15: "h"}
            # norm emission lags the stripe by enough dots to cover the
            # square engine's latency (PE executes in order: a norm matmul
            # waiting on a late square stalls every matmul behind it)
            SQ_LAG = {"g": 4, "a": 2, "v": 2, "h": 1}
            pending = []
            dotPs = {}
            nrmPs = {}

            def emit_nrms(p):
                pb, ps, mode, sq = p
                pj = pb // 2
                rows = slice(32 * pj, 32 * pj + 32)
                half = NW * (pb % 2)
                first = (pb % 2 == 0) and (ps == 0)
                last = (pb % 2 == 1) and (ps == NST - 1)
                ndr = {"v": 0, "h": SW // DRW // 2}.get(mode, SW // DRW)
                for tl in range(ndr):      # fp8 DoubleRow windows
                    td = ps * (SW // DRW) + tl
                    o = 127 - (32 * pj + half + 2 * td)
                    nc.tensor.matmul(
                        nrmPs[pj][:, :],
                        lhsT=strip2[:, :, o:o + 128],
                        rhs=sq[:, tl * DRW:(tl + 1) * DRW].rearrange(
                            "p (k f) -> p k f", k=2),
                        perf_mode=DR,
                        start=first and (tl == 0),
                        stop=(mode != "v") and last and (tl == SW // DRW - 1)
                             and mode != "h",
                        skip_group_check=True,
                        tile_position=(0, 0),
                    )
                if mode in ("v", "h"):     # bf16 windows (squared in place)
                    st = stripes.pop((pb, ps))
                    t0 = 0 if mode == "v" else SW // W // 2
                    for tl in range(t0, SW // W):
                        t = ps * (SW // W) + tl
                        c = half + t
                        nc.tensor.matmul(
                            nrmPs[pj][rows, :],
                            lhsT=ones_strip[:, 31 - c:63 - c],
                            rhs=st[:, tl * W:(tl + 1) * W],
                            start=False,
                            stop=last and (tl == SW // W - 1),
                            skip_group_check=True,
                            tile_position=(0, 32 * pj),
                        )
                if last:
                    nc.vector.tensor_copy(NS_sb[rows, :], nrmPs[pj][rows, :])

            for b in range(BL):
                j = b // 2
                if b % 2 == 0:
                    dotPs[j] = psA.tile([128, W], F32, tag="dotP", name=f"dotP_{j}")
                    nrmPs[j] = psA.tile([128, W], F32, tag="nrmP", bufs=3, name=f"nrmP_{j}")
                for s in range(NST):
                    if (b, s) in stripes:
                        st = stripes[(b, s)]
                    else:
                        st = memp.tile([128, SW], BF16, tag="mst", name=f"mst_{b}_{s}")
                        nc.sync.dma_start(st[:], memT_ap[b][:, s * SW:(s + 1) * SW])
                        stripes[(b, s)] = st
                    rows = slice(32 * j, 32 * j + 32)
                    for tl in range(SW // W):
                        t = s * (SW // W) + tl
                        c = NW * (b % 2) + t
                        nc.tensor.matmul(
                            dotPs[j][rows, :],
                            lhsT=strips[:, b, 31 - c:63 - c],
                            rhs=st[:, tl * W:(tl + 1) * W],
                            start=(b % 2 == 0) and (t == 0),
                            stop=(b % 2 == 1) and (t == NW - 1),
                            skip_group_check=True,
                            tile_position=(0, 32 * j),
                        )
                    if (b % 2 == 1) and (s == NST - 1):
                        # dots of this group are done; free the PSUM slot now
                        nc.vector.tensor_copy(D_sb[rows, :], dotPs[j][rows, :])
                    e = SQ_ENG.get(b * NST + s, "a")
                    sq = None
                    if e == "v":
                        nc.vector.tensor_mul(st[:], st[:], st[:])
                    elif e == "h":
                        # endgame: first half -> fp8 on ACT, second half
                        # squared in place on DVE; both engines finish ~2us
                        # after the stripe lands.
                        sq = sqp.tile([128, SW // 2], F8, tag="sq8h", bufs=2,
                                      name=f"sq_{b}_{s}")
                        nc.scalar.activation(sq[:], st[:, 0:SW // 2], AF.Square)
                        nc.vector.tensor_mul(st[:, SW // 2:SW],
                                             st[:, SW // 2:SW],
                                             st[:, SW // 2:SW])
                    else:
                        sq = sqp.tile([128, SW], F8, tag="sq8", name=f"sq_{b}_{s}")
                        if e == "a":
                            nc.scalar.activation(sq[:], st[:], AF.Square)
                        else:
                            nc.gpsimd.tensor_mul(sq[:], st[:], st[:])
                        stripes.pop((b, s))
                    pending.append((b, s, e, sq))
                    while pending:
                        idx = b * NST + s
                        h0 = pending[0]
                        hidx = h0[0] * NST + h0[1]
                        if idx - hidx >= SQ_LAG[h0[2]]:
                            emit_nrms(pending.pop(0))
                        else:
                            break
            for p in pending:
                emit_nrms(p)

            # ---- light phase tail ----
            emit_simE(0)
            emit_simE(1)
            S_ps = psB.tile([BL, 1], F32, tag="tiny")
            nc.tensor.matmul(S_ps[:], lhsT=ind_sb, rhs=rs1[:], start=True, stop=True)
            rS = lp.tile([BL, 1], F32)
            nc.vector.reciprocal(rS[:], S_ps[:])
            gs = lp.tile([BL, 1], F32)
            nc.vector.tensor_mul(gs[:], gate[:], rS[:])
            F2_ps = psB.tile([128, 1], F32, tag="tiny")
            nc.tensor.matmul(F2_ps[:], lhsT=indT_sb, rhs=gs[:], start=True, stop=True)
            F_gs = lp.tile([128, 1], F32)
            nc.vector.tensor_copy(F_gs[:], F2_ps[:])

            # gated = gs*E + (1-gate)*pw   (gs = gate/softmax_sum; t4 ready)
            Esc = lp.tile([128, 1], F32)
            nc.vector.tensor_copy(Esc[:], E[:, 0:1])  # DVE observes ACT@E
            G = lp.tile([128, W], F32)
            nc.vector.scalar_tensor_tensor(
                G[:], E[:], F_gs[:], t4[:], op0=OP.mult, op1=OP.add
            )

            # circular conv: SH = s1*G + s0*roll(G,-1) + s2*roll(G,+1)
            SH = lp.tile([128, W], F32)
            nc.vector.tensor_scalar_mul(SH[:], G[:], F_s1)
            nc.vector.scalar_tensor_tensor(
                SH[:, 0:W - 1], G[:, 1:W], F_s0, SH[:, 0:W - 1],
                op0=OP.mult, op1=OP.add,
            )
            nc.vector.scalar_tensor_tensor(
                SH[:, 1:W], G[:, 0:W - 1], F_s2, SH[:, 1:W],
                op0=OP.mult, op1=OP.add,
            )
            # boundary columns: SH[p, W-1] += s0 * G[p+1 (wrap in batch), 0]
            # and SH[p, 0] += s2 * G[p-1 (wrap), W-1].  One bf16 matmul per
            # direction against a constant in-batch shift permutation does
            # the partition shift (out[m] = sum_p perm[p, m] * G[p, col]).
            Gb = lp.tile([128, 2], BF16)
            nc.vector.tensor_copy(Gb[:, 0:1], G[:, 0:1])
            nc.vector.tensor_copy(Gb[:, 1:2], G[:, W - 1:W])
            bl_ps = psB.tile([128, 1], F32, tag="tiny")
            nc.tensor.matmul(bl_ps[:], lhsT=shifts_bf[:, 0:128], rhs=Gb[:, 0:1],
                             start=True, stop=True)
            bl = lp.tile([128, 1], F32)
            nc.vector.tensor_copy(bl[:], bl_ps[:])
            nc.vector.scalar_tensor_tensor(
                SH[:, W - 1:W], bl[:], F_s0, SH[:, W - 1:W],
                op0=OP.mult, op1=OP.add,
            )
            br_ps = psB.tile([128, 1], F32, tag="tiny")
            nc.tensor.matmul(br_ps[:], lhsT=shifts_bf[:, 128:256], rhs=Gb[:, 1:2],
                             start=True, stop=True)
            br = lp.tile([128, 1], F32)
            nc.vector.tensor_copy(br[:], br_ps[:])
            nc.vector.scalar_tensor_tensor(
                SH[:, 0:1], br[:], F_s2, SH[:, 0:1], op0=OP.mult, op1=OP.add
            )

            # sharpening: P2 = (SH + 1e-8)^gamma = exp(gamma * ln(SH + 1e-8))
            Lg = lp.tile([128, W], F32)
            nc.scalar.activation(Lg[:], SH[:], AF.Ln, bias=eps_col)
            P2 = lp.tile([128, W], F32)
            rs2 = lp.tile([128, 1], F32)
            nc.scalar.activation(P2[:], Lg[:], AF.Exp, scale=F_gamma, accum_out=rs2[:])
            S2_ps = psB.tile([BL, 1], F32, tag="tiny")
            nc.tensor.matmul(S2_ps[:], lhsT=ind_sb, rhs=rs2[:], start=True, stop=True)
            S2 = lp.tile([BL, 1], F32)
            nc.vector.tensor_scalar_add(S2[:], S2_ps[:], EPS)
            r2 = lp.tile([BL, 1], F32)
            nc.vector.reciprocal(r2[:], S2[:])
            F3_ps = psB.tile([128, 1], F32, tag="tiny")
            nc.tensor.matmul(F3_ps[:], lhsT=indT_sb, rhs=r2[:], start=True, stop=True)
            F_r2 = lp.tile([128, 1], F32)
            nc.vector.tensor_copy(F_r2[:], F3_ps[:])

            P2sc = lp.tile([128, 1], F32)
            nc.vector.tensor_copy(P2sc[:], P2[:, 0:1])  # DVE observes ACT@P2
            outsb = lp.tile([128, W], F32)
            nc.vector.tensor_scalar_mul(outsb[:], P2[:], F_r2[:])
            nc.sync.dma_start(
                out_d[:].rearrange("b (q f) -> (b q) f", f=W), outsb[:]
            )
    nc.compile()
    return nc


def _get_nc():
    global _NC
    if _NC is None:
        _NC = build_nc()
    return _NC


def _enable_profiling():
    """Install the axon NTFF profile hook; the agent image lacks
    antenv.axon_hooks, so shim it and register the ctypes-based hook."""
    import types

    import concourse.bass_utils as bu

    bu.upload_artifacts = lambda tmpdir: tmpdir  # no artifact bucket here
    try:
        from antenv.axon_hooks import get_axon_ntff_profile_hook  # noqa: F401

        return
    except ImportError:
        pass
    import antenv

    mod = types.ModuleType("antenv.axon_hooks")
    _holder = {}
    mod.set_axon_ntff_profile_hook = lambda h: _holder.__setitem__("h", h)
    mod.get_axon_ntff_profile_hook = lambda: _holder.get("h")
    sys.modules["antenv.axon_hooks"] = mod
    antenv.axon_hooks = mod
    from trn_agent_boot.trn_boot import _ntff_profile_via_ctypes

    mod.set_axon_ntff_profile_hook(
        _ntff_profile_via_ctypes("/opt/axon/libaxon_pjrt.so")
    )


def kernel(**inputs):
    global LAST_RESULTS
    mem = np.ascontiguousarray(np.asarray(inputs["memory"], dtype=np.float32))
    cs = np.ascontiguousarray(np.asarray(inputs["controller_state"], dtype=np.float32))
    pw = np.ascontiguousarray(np.asarray(inputs["previous_weights"], dtype=np.float32))
    Wk = np.asarray(inputs["Wk"], np.float32)
    Wcat = np.concatenate(
        [
            np.asarray(inputs["Wb"], np.float32),
            np.asarray(inputs["Wg"], np.float32),
            np.asarray(inputs["Ws"], np.float32),
            np.asarray(inputs["Wgam"], np.float32),
        ],
        axis=1,
    )
    brow = np.concatenate(
        [
            np.asarray(inputs["bb"], np.float32),
            np.asarray(inputs["bg"], np.float32),
            np.asarray(inputs["bs"], np.float32),
            np.asarray(inputs["bgam"], np.float32),
        ]
    )

    # shard: core c gets batches [c*BL, (c+1)*BL); memory pre-transposed to (BL, D, N)
    memT = np.ascontiguousarray(
        mem.reshape(NCORES, BL, N, D).transpose(0, 1, 3, 2)
    )
    import ml_dtypes
    memT = memT.astype(ml_dtypes.bfloat16)
    csT = cs.reshape(NCORES, BL, C).transpose(0, 2, 1)  # (cores, C, BL)
    pw_sh = pw.reshape(NCORES, BL, N)

    smalls = np.zeros((NCORES, 128, SC), np.float32)
    smalls[:, :, S_CST:S_CST + BL] = csT[:, 0:128, :]
    smalls[:, :, S_CST + BL:S_CST + 2 * BL] = csT[:, 128:256, :]
    smalls[:, :, S_WK:S_WK + D] = Wk[0:128, :]
    smalls[:, :, S_WK + D:S_WK + 2 * D] = Wk[128:256, :]
    smalls[:, :, S_WC:S_WC + 6] = Wcat[0:128, :]
    smalls[:, :, S_WC + 6:S_WC + 12] = Wcat[128:256, :]
    smalls[:, 0:BL, S_B6:S_B6 + 6] = brow[None, None, :]

    in_maps = [
        {
            "memT": memT[c],
            "smalls": np.ascontiguousarray(smalls[c]),
            "pw": np.ascontiguousarray(pw_sh[c]),
        }
        for c in range(NCORES)
    ]
    nc = _get_nc()
    if PROFILE:
        _enable_profiling()
    res = run_bass_kernel_spmd(nc, in_maps, list(range(NCORES)), trace=PROFILE)
    LAST_RESULTS = res
    out = np.concatenate([r["out"] for r in res.results], axis=0)
    return out.astype(np.float32)


# revision 49
# speedup vs baseline: 1.1386x; 1.0569x over previous
"""Trainium2 Bass kernel for a differentiable addressing head (NTM-style).

Computes, for each batch b:
    key   = cs @ Wk;  beta = softplus(cs@Wb+bb)+1;  gate = sigmoid(cs@Wg+bg)
    shift = softmax(cs@Ws+bs);  gamma = softplus(cs@Wgam+bgam)+1
    sim   = (key . mem[n]) / (|key||mem[n]| + eps)
    cw    = softmax(beta * sim);  g = gate*cw + (1-gate)*pw
    sh    = circular_conv(g, shift);  w = (sh+1e-8)^gamma / (sum + eps)

Sharding: data-parallel over batch across 8 cores (8 batches/core).

Heavy phase per core: one pass over memory (16 MB bf16) computing dot
products on the PE (bf16, 1 col/cycle) and row norms via fp8 DoubleRow
matmuls (0.5 cyc/col) over on-chip-squared fp8 data. The DoubleRow k-tile
split is just an AP view (two contiguous 512-col halves of a 1024-col
window), so everything stays in the natural light layout.

Startup: all small tensors ride in 2 packed DMAs ahead of the memory
stripes, and a warm-up matmul stream keeps the PE busy (and ramps its
p-state to 2.4 GHz) while the first stripe is in flight.

Self-contained: hardcodes shapes B=64, N=8192, D=128, C=256.
"""

import os
import sys

import numpy as np

for _p in ("/opt/trn_rl_repo", "/opt/pypackages"):
    if _p not in sys.path and os.path.isdir(_p):
        sys.path.insert(0, _p)

import concourse.bacc as bacc
import concourse.bass as bass
import concourse.tile as tile
from concourse import mybir
from concourse.bass_utils import run_bass_kernel_spmd

F32 = mybir.dt.float32
BF16 = mybir.dt.bfloat16
F8 = mybir.dt.float8e4        # TRN e4m3 (max 240)
AF = mybir.ActivationFunctionType
OP = mybir.AluOpType
DR = mybir.MatmulPerfMode.DoubleRow

B, N, D, C = 64, 8192, 128, 256
NCORES = 8
BL = B // NCORES          # batches per core = 8
NW = 16                   # 512-wide blocks per batch row
W = N // NW               # 512, block width (= light-tile free dim)
NST = 2                   # DMA stripes per batch
SW = N // NST             # 4096, stripe width
DRW = 2 * W               # 1024, DoubleRow norm window (2 blocks)
EPS = 1e-8
NWARM = 10                # PE warm-up matmuls

# packed-constant column offsets (fp32, 128 partitions)
C_IND = 0                 # ind (128, 8)
C_INDT = 8                # indT padded (8 rows used) 128
C_ONES = 136              # ones col
C_ONE1 = 137              # 1x1 one (row 0)
C_EPS = 138               # eps col
C_S2 = 139                # strip2 (2 planes x 256)
C_OS = C_S2 + 512         # ones_strip (63)
C_SU = C_OS + 63          # shift-up perm (128): row m <- row (m+1 wrap in batch)
C_SD = C_SU + 128         # shift-down perm (128)
CC = C_SD + 128           # 970 cols

# packed-smalls column offsets (fp32, 128 partitions)
S_CST = 0                 # csT two 128-blocks (2*8)
S_WK = 16                 # Wk two 128-blocks (2*128)
S_WC = 272                # Wcat two 128-blocks (2*6)
S_B6 = 284                # bias6 (8 rows used)
SC = S_B6 + 6             # 290 cols

# Module-level caches so repeat calls don't rebuild/recompile.
_NC = None
PROFILE = False
LAST_RESULTS = None       # BassKernelResults of the last run (for profiling)


def _const_pack():
    cp = np.zeros((128, CC), np.float32)
    for p in range(128):
        cp[p, C_IND + p // NW] = 1.0          # ind[p, b] = 1 iff p//16 == b
    # indT[b, p] = 1 iff p//16 == b  (rows 0..7)
    for b in range(BL):
        for p in range(128):
            if p // NW == b:
                cp[b, C_INDT + p] = 1.0
    cp[:, C_ONES] = 1.0
    cp[0, C_ONE1] = 1.0
    cp[:, C_EPS] = EPS
    # strip2[p, k, c] = 1 iff c == 127 + k ; sliced at offset 127 - row the
    # DoubleRow matmul lands the two k-plane column sums on rows (r, r+1).
    # (the ISA requires DoubleRow weights to be full 128 columns wide)
    cp[:, C_S2 + 0 * 256 + 127] = 1.0
    cp[:, C_S2 + 1 * 256 + 128] = 1.0
    cp[:, C_OS + 31] = 1.0    # bf16 ones strip, col 31 (norms of bf16 stripes)
    # circ-conv boundary shifts: out[m] = in[perm(m)], perm wraps inside each
    # batch's 16-row block (partition p = b*16 + t).  matmul out[m] =
    # sum_p lhsT[p, m] * in[p]  =>  lhsT[p, m] = 1 iff p == perm(m).
    for m in range(128):
        b, t = m // NW, m % NW
        cp[b * NW + (t + 1) % NW, C_SU + m] = 1.0
        cp[b * NW + (t - 1) % NW, C_SD + m] = 1.0
    return cp


def _patch_act_tables():
    """Prefer the combined natural_log_exp table set. The set chooser maps
    each activation to the FIRST set containing its func, so exp->set0 and
    ln->natural_log oscillate, costing a 1.3us table load per transition on
    the critical path. natural_log_exp_and_others covers every func this
    kernel uses (exp, ln, square, copy, identity) -> one load total."""
    import concourse.hw_specs as hw_specs

    if getattr(bacc, "_act_tables_patched", False):
        return
    orig = bacc.get_activation_tables

    def filtered(module_arch):
        t = orig(module_arch)
        pref = "natural_log_exp_and_others"
        if pref in t:
            mine = {
                AF.Exp, AF.Ln, AF.Square, AF.Copy, AF.Identity, AF.MemsetZero
            } & t[pref]
            # keep dict ORDER (ids are positional); only shrink other sets
            for k in t:
                if k != pref:
                    t[k] = t[k] - mine
        return t

    bacc.get_activation_tables = filtered
    bacc._act_tables_patched = True


def build_nc():
    # Bacc (not plain Bass): its compile() runs generate_event_semaphores,
    # which splits multi-wait sync conditions to satisfy the TRN2 limit of
    # one sync wait per instruction.
    _patch_act_tables()
    nc = bacc.Bacc()

    memT_d = nc.dram_tensor("memT", [BL, D, N], BF16, kind="ExternalInput")
    smalls_d = nc.dram_tensor("smalls", [128, SC], F32, kind="ExternalInput")
    pw_d = nc.dram_tensor("pw", [BL, N], F32, kind="ExternalInput")
    out_d = nc.dram_tensor("out", [BL, N], F32, kind="ExternalOutput")
    cpack_c = nc.inline_tensor(_const_pack(), "cpack_c")

    with tile.TileContext(nc) as tc:
        with (
            tc.tile_pool(name="const", bufs=1) as cp,
            tc.tile_pool(name="mem", bufs=7) as memp,
            tc.tile_pool(name="sq", bufs=3) as sqp,
            tc.tile_pool(name="light", bufs=1) as lp,
            tc.tile_pool(name="psmm", bufs=2, space="PSUM") as psA,
            tc.tile_pool(name="pstiny", bufs=2, space="PSUM") as psB,
        ):
            # ---- packed small DMAs first: nothing big ahead of them ----
            smalls_raw = cp.tile([128, SC], F32, name="smalls_raw")
            nc.sync.dma_start(smalls_raw[:], smalls_d[:])
            cpack_raw = cp.tile([128, CC], F32, name="cpack_raw")
            nc.sync.dma_start(cpack_raw[:], cpack_c[:])

            # ---- GPSIMD staging (single producer proc for matmul inputs;
            # self-loading fp32 matmuls support only ONE sync wait, so all
            # const reads must come from one engine whose newest tick the PE
            # absorbs once). warm is staged FIRST so the PE can start early;
            # smalls_g is staged LAST so one wait covers every gpsimd tick.
            warm = cp.tile([128, 288], BF16, name="warm")
            nc.gpsimd.tensor_copy(warm[:], smalls_raw[:, 0:288])
            cpack = cp.tile([128, CC], F32, name="cpack")
            nc.gpsimd.tensor_copy(cpack[:], cpack_raw[:])
            ones_strip = cp.tile([128, 63], BF16, name="ones_strip")
            nc.gpsimd.tensor_copy(ones_strip[:], cpack_raw[:, C_OS:C_OS + 63])
            shifts_bf = cp.tile([128, 256], BF16, name="shifts_bf")
            nc.gpsimd.tensor_copy(shifts_bf[:], cpack_raw[:, C_SU:C_SU + 256])
            smalls = cp.tile([128, SC], F32, name="smalls")
            nc.gpsimd.tensor_copy(smalls[:], smalls_raw[:])

            ind_sb = cpack[:, C_IND:C_IND + BL]
            indT_sb = cpack[0:BL, C_INDT:C_INDT + 128]
            ones_col = cpack[:, C_ONES:C_ONES + 1]
            one1 = cpack[0:1, C_ONE1:C_ONE1 + 1]
            eps_col = cpack[:, C_EPS:C_EPS + 1]
            csT0 = smalls[:, S_CST:S_CST + BL]
            csT1 = smalls[:, S_CST + BL:S_CST + 2 * BL]
            Wk0 = smalls[:, S_WK:S_WK + D]
            Wk1 = smalls[:, S_WK + D:S_WK + 2 * D]
            Wc0 = smalls[:, S_WC:S_WC + 6]
            Wc1 = smalls[:, S_WC + 6:S_WC + 12]
            bias6 = smalls[0:BL, S_B6:S_B6 + 6]

            # ---- PE warm-up: keeps the PE busy (and its p-state ramping
            # toward 2.4 GHz) while the first memory stripe is in flight.
            # Write-only absorber output; also absorbs the warm gpsimd tick.
            wps = psB.tile([32, 288], F32, tag="absorb", bufs=1, name="wps")
            for i in range(NWARM):
                nc.tensor.matmul(wps[:], lhsT=warm[:, 0:32], rhs=warm[:],
                                 start=True, stop=True, skip_group_check=True)

            # ---- memory stripe prefetch + pw ----
            memT_ap = memT_d[:]
            stripes = {}
            for pb, ps in ((0, 0), (0, 1), (1, 0)):
                st = memp.tile([128, SW], BF16, tag="mst", name=f"mst_{pb}_{ps}")
                nc.sync.dma_start(st[:], memT_ap[pb][:, ps * SW:(ps + 1) * SW])
                stripes[(pb, ps)] = st
            pw_raw = cp.tile([128, W], F32)
            nc.sync.dma_start(pw_raw[:], pw_d[:].rearrange("b (q f) -> (b q) f", f=W))

            # ---- projections: key_T (D, BL) and proj (BL, 6) ----
            # (the first matmul's single wait on smalls' gpsimd tick also
            # absorbs every earlier staging op into the PE's clock)
            key_ps = psB.tile([128, BL], F32, tag="tiny")
            nc.tensor.matmul(key_ps[:], lhsT=Wk0, rhs=csT0, start=True, stop=False)
            nc.tensor.matmul(key_ps[:], lhsT=Wk1, rhs=csT1, start=False, stop=True)
            proj_ps = psB.tile([BL, 6], F32, tag="tiny")
            nc.tensor.matmul(proj_ps[:], lhsT=csT0, rhs=Wc0, start=True, stop=False)
            nc.tensor.matmul(proj_ps[:], lhsT=csT1, rhs=Wc1, start=False, stop=True)

            keyT = cp.tile([128, BL], F32)
            proj = lp.tile([BL, 6], F32)
            # DVE proc: strip2 staged first, then keyT/strips; one PE
            # absorber on strips then covers every DVE tick.
            strip2 = cp.tile([128, 2, 256], F8, name="strip2")
            nc.vector.tensor_copy(
                strip2[:], cpack_raw[:, C_S2:C_S2 + 512].rearrange(
                    "p (k c) -> p k c", k=2)
            )
            nc.vector.tensor_copy(keyT[:], key_ps[:])
            nc.vector.tensor_add(proj[:], proj_ps[:], bias6)
            pw_sb = cp.tile([128, W], F32)
            nc.vector.tensor_copy(pw_sb[:], pw_raw[:])

            # zero-padded key strips: strips[:, b, 31] = key_T[:, b]
            strips = cp.tile([128, BL, 63], BF16)
            nc.vector.memset(strips[:], 0.0)
            for b in range(BL):
                nc.vector.tensor_copy(strips[:, b, 31:32], keyT[:, b:b + 1])

            # absorb the newest DVE tick (strips) into the PE clock so the
            # dot matmuls carry only their stripe-DMA wait.
            nc.tensor.matmul(wps[0:BL, 0:1], lhsT=strips[:, :, 31],
                             rhs=strips[:, 0, 31:32],
                             start=True, stop=True, skip_group_check=True)

            # |key|^2 per batch -> (BL, 1), broadcast to partitions
            kq = lp.tile([128, BL], F32)
            nc.scalar.activation(kq[:], keyT[:], AF.Square)
            kn2_ps = psB.tile([BL, 1], F32, tag="tiny")
            nc.tensor.matmul(kn2_ps[:], lhsT=kq[:], rhs=ones_col, start=True, stop=True)
            kn2 = lp.tile([BL, 1], F32)
            nc.vector.tensor_copy(kn2[:], kn2_ps[:])
            kn2F_ps = psB.tile([128, 1], F32, tag="tiny")
            nc.tensor.matmul(kn2F_ps[:], lhsT=indT_sb, rhs=kn2[:], start=True, stop=True)
            F_kn2 = lp.tile([128, 1], F32)
            nc.vector.tensor_copy(F_kn2[:], kn2F_ps[:])

            # per-batch scalars: beta, 1-gate, s0, s1, s2, gamma  (BL, 6);
            # emitted BEFORE the heavy loop so FB and t4 are ready when the
            # row-split sim/E passes fire mid-phase.
            scal = lp.tile([BL, 6], F32)
            # softplus(x) = ln(1 + exp(x)); beta = softplus + 1
            eb = lp.tile([BL, 1], F32)
            nc.scalar.activation(eb[:], proj[:, 0:1], AF.Exp)
            sp_b = lp.tile([BL, 1], F32)
            nc.scalar.activation(sp_b[:], eb[:], AF.Ln, bias=1.0)
            nc.vector.tensor_scalar_add(scal[:, 0:1], sp_b[:], 1.0)
            # gate = sigmoid(x) = 1 / (1 + exp(-x))
            eg = lp.tile([BL, 1], F32)
            nc.scalar.activation(eg[:], proj[:, 1:2], AF.Exp, scale=-1.0)
            dg = lp.tile([BL, 1], F32)
            nc.vector.tensor_scalar_add(dg[:], eg[:], 1.0)
            gate = lp.tile([BL, 1], F32)
            nc.vector.reciprocal(gate[:], dg[:])
            nc.vector.tensor_scalar(
                scal[:, 1:2], gate[:], -1.0, 1.0, op0=OP.mult, op1=OP.add
            )
            e3 = lp.tile([BL, 3], F32)
            nc.scalar.activation(e3[:], proj[:, 2:5], AF.Exp)
            ssum = lp.tile([BL, 1], F32)
            nc.vector.reduce_sum(ssum[:], e3[:], axis=mybir.AxisListType.X)
            rssum = lp.tile([BL, 1], F32)
            nc.vector.reciprocal(rssum[:], ssum[:])
            sh3 = lp.tile([BL, 3], F32)
            nc.scalar.mul(sh3[:], e3[:], rssum[:])
            nc.vector.tensor_copy(scal[:, 2:5], sh3[:])
            # gamma = softplus(z) + 1 = ln(1 + exp(z)) + 1
            egm = lp.tile([BL, 1], F32)
            nc.scalar.activation(egm[:], proj[:, 5:6], AF.Exp)
            sp_g = lp.tile([BL, 1], F32)
            nc.scalar.activation(sp_g[:], egm[:], AF.Ln, bias=1.0)
            nc.vector.tensor_scalar_add(scal[:, 5:6], sp_g[:], 1.0)
            # broadcast to per-partition fields (128, 6)
            FB_ps = psB.tile([128, 6], F32, tag="tiny")
            nc.tensor.matmul(FB_ps[:], lhsT=indT_sb, rhs=scal[:], start=True, stop=True)
            FB = lp.tile([128, 6], F32)
            nc.vector.tensor_copy(FB[:], FB_ps[:])
            F_beta = FB[:, 0:1]
            F_g1 = FB[:, 1:2]
            F_s0 = FB[:, 2:3]
            F_s1 = FB[:, 3:4]
            F_s2 = FB[:, 4:5]
            F_gamma = FB[:, 5:6]
            t4 = lp.tile([128, W], F32)
            nc.vector.tensor_scalar_mul(t4[:], pw_sb[:], F_g1)

            # ---- heavy phase: dot[b, n] and normsq[b, n] ----
            # Per stripe: 8 bf16 dot matmuls (512-col windows, shifted-strip
            # diag trick packs block t of batch pair into PSUM row 16*(b%2)+t),
            # then the stripe is squared into a separate fp8 tile (chunks
            # rotate over DVE/ACT/GPSIMD), then the PREVIOUS stripe's 4
            # DoubleRow norm matmuls run (fp8, 2 PSUM rows per 1024-col
            # window; the k-tile split is the two 512-col halves).
            D_sb = lp.tile([128, W], F32)    # dot, light layout (p = b*16+t, f)
            NS_sb = lp.tile([128, W], F32)   # |mem|^2, light layout
            # sim/E pipeline tiles, written row-half at a time as batch-pair
            # groups complete (rows 0:64 mid-phase, 64:128 at the end) so
            # most of the serial tail chain overlaps the heavy phase.
            Lv = lp.tile([128, W], F32)
            y1 = lp.tile([128, W], F32)
            sim = lp.tile([128, W], F32)
            E = lp.tile([128, W], F32)
            rs1 = lp.tile([128, 1], F32)

            def emit_simE(hh):
                r = slice(64 * hh, 64 * hh + 64)
                # 1/(kn*mn) = exp(-0.5*ln(kn2*ns)); then E = exp(beta*sim)
                nc.scalar.activation(Lv[r, :], NS_sb[r, :], AF.Ln, scale=F_kn2[r, :])
                nc.scalar.activation(y1[r, :], Lv[r, :], AF.Exp, scale=-0.5)
                nc.vector.tensor_mul(sim[r, :], D_sb[r, :], y1[r, :])
                nc.scalar.activation(E[r, :], sim[r, :], AF.Exp,
                                     scale=F_beta[r, :], accum_out=rs1[r, :])
            # per-stripe square engine: "a"/"g" square to a separate fp8 tile
            # (DoubleRow norms, 0.21 ns/col on PE); "v" squares IN PLACE in
            # bf16 on the DVE 2x path (0.86 ns/col) and pays bf16-rate norm
            # matmuls instead -- the PE has headroom, ACT/DVE don't.
            # Group-leading stripes (0,4,8,12) must be fp8 so the group's
            # first norm matmul covers all 128 PSUM rows (start=True zeroing).
            # GPSIMD squares are slow (7.3us/stripe) -- schedule them EARLY;
            # the last batch's stripes split into half-stripe chunks spread
            # over ACT+DVE so the endgame squares finish ~2us after the DMA.
            SQ_ENG = {3: "v", 7: "v", 11: "v", 2: "g", 8: "g",
                      14: "h", 15: "h"}
            # norm emission lags the stripe by enough dots to cover the
            # square engine's latency (PE executes in order: a norm matmul
            # waiting on a late square stalls every matmul behind it)
            SQ_LAG = {"g": 4, "a": 2, "v": 3, "h": 1}
            pending = []
            dotPs = {}
            nrmPs = {}

            def emit_nrms(p):
                pb, ps, mode, sq = p
                pj = pb // 2
                rows = slice(32 * pj, 32 * pj + 32)
                half = NW * (pb % 2)
                first = (pb % 2 == 0) and (ps == 0)
                last = (pb % 2 == 1) and (ps == NST - 1)
                ndr = {"v": 0, "h": SW // DRW // 2}.get(mode, SW // DRW)
                for tl in range(ndr):      # fp8 DoubleRow windows
                    td = ps * (SW // DRW) + tl
                    o = 127 - (32 * pj + half + 2 * td)
                    nc.tensor.matmul(
                        nrmPs[pj][:, :],
                        lhsT=strip2[:, :, o:o + 128],
                        rhs=sq[:, tl * DRW:(tl + 1) * DRW].rearrange(
                            "p (k f) -> p k f", k=2),
                        perf_mode=DR,
                        start=first and (tl == 0),
                        stop=(mode != "v") and last and (tl == SW // DRW - 1)
                             and mode != "h",
                        skip_group_check=True,
                        tile_position=(0, 0),
                    )
                if mode in ("v", "h"):     # bf16 windows (squared in place)
                    st = stripes.pop((pb, ps))
                    t0 = 0 if mode == "v" else SW // W // 2
                    for tl in range(t0, SW // W):
                        t = ps * (SW // W) + tl
                        c = half + t
                        nc.tensor.matmul(
                            nrmPs[pj][rows, :],
                            lhsT=ones_strip[:, 31 - c:63 - c],
                            rhs=st[:, tl * W:(tl + 1) * W],
                            start=False,
                            stop=last and (tl == SW // W - 1),
                            skip_group_check=True,
                            tile_position=(0, 32 * pj),
                        )
                if last:
                    nc.vector.tensor_copy(NS_sb[rows, :], nrmPs[pj][rows, :])

            for b in range(BL):
                j = b // 2
                if b % 2 == 0:
                    dotPs[j] = psA.tile([128, W], F32, tag="dotP", name=f"dotP_{j}")
                    nrmPs[j] = psA.tile([128, W], F32, tag="nrmP", bufs=3, name=f"nrmP_{j}")
                for s in range(NST):
                    if (b, s) in stripes:
                        st = stripes[(b, s)]
                    else:
                        st = memp.tile([128, SW], BF16, tag="mst", name=f"mst_{b}_{s}")
                        nc.sync.dma_start(st[:], memT_ap[b][:, s * SW:(s + 1) * SW])
                        stripes[(b, s)] = st
                    rows = slice(32 * j, 32 * j + 32)
                    for tl in range(SW // W):
                        t = s * (SW // W) + tl
                        c = NW * (b % 2) + t
                        nc.tensor.matmul(
                            dotPs[j][rows, :],
                            lhsT=strips[:, b, 31 - c:63 - c],
                            rhs=st[:, tl * W:(tl + 1) * W],
                            start=(b % 2 == 0) and (t == 0),
                            stop=(b % 2 == 1) and (t == NW - 1),
                            skip_group_check=True,
                            tile_position=(0, 32 * j),
                        )
                    if (b % 2 == 1) and (s == NST - 1):
                        # dots of this group are done; free the PSUM slot now
                        nc.vector.tensor_copy(D_sb[rows, :], dotPs[j][rows, :])
                    e = SQ_ENG.get(b * NST + s, "a")
                    sq = None
                    if e == "v":
                        nc.vector.tensor_mul(st[:], st[:], st[:])
                    elif e == "h":
                        # endgame: first half -> fp8 on ACT, second half
                        # squared in place on DVE; both engines finish ~2us
                        # after the stripe lands.
                        sq = sqp.tile([128, SW // 2], F8, tag="sq8h", bufs=2,
                                      name=f"sq_{b}_{s}")
                        nc.scalar.activation(sq[:], st[:, 0:SW // 2], AF.Square)
                        nc.vector.tensor_mul(st[:, SW // 2:SW],
                                             st[:, SW // 2:SW],
                                             st[:, SW // 2:SW])
                    else:
                        sq = sqp.tile([128, SW], F8, tag="sq8", name=f"sq_{b}_{s}")
                        if e == "a":
                            nc.scalar.activation(sq[:], st[:], AF.Square)
                        else:
                            nc.gpsimd.tensor_mul(sq[:], st[:], st[:])
                        stripes.pop((b, s))
                    pending.append((b, s, e, sq))
                    if b * NST + s == 14:
                        # rows 0:64 (groups 0-1) are long done -- run their
                        # sim/E chain now so only rows 64:128 remain for the
                        # tail.  Emitted here (late) so it cannot head-of-line
                        # block earlier squares on the ACT queue.
                        emit_simE(0)
                    while pending:
                        idx = b * NST + s
                        h0 = pending[0]
                        hidx = h0[0] * NST + h0[1]
                        if idx - hidx >= SQ_LAG[h0[2]]:
                            emit_nrms(pending.pop(0))
                        else:
                            break
            for p in pending:
                emit_nrms(p)

            # ---- light phase tail ----
            emit_simE(1)
            S_ps = psB.tile([BL, 1], F32, tag="tiny")
            nc.tensor.matmul(S_ps[:], lhsT=ind_sb, rhs=rs1[:], start=True, stop=True)
            rS = lp.tile([BL, 1], F32)
            nc.vector.reciprocal(rS[:], S_ps[:])
            gs = lp.tile([BL, 1], F32)
            nc.vector.tensor_mul(gs[:], gate[:], rS[:])
            F2_ps = psB.tile([128, 1], F32, tag="tiny")
            nc.tensor.matmul(F2_ps[:], lhsT=indT_sb, rhs=gs[:], start=True, stop=True)
            F_gs = lp.tile([128, 1], F32)
            nc.vector.tensor_copy(F_gs[:], F2_ps[:])

            # gated = gs*E + (1-gate)*pw   (gs = gate/softmax_sum; t4 ready)
            Esc = lp.tile([128, 1], F32)
            nc.vector.tensor_copy(Esc[:], E[:, 0:1])  # DVE observes ACT@E
            G = lp.tile([128, W], F32)
            nc.vector.scalar_tensor_tensor(
                G[:], E[:], F_gs[:], t4[:], op0=OP.mult, op1=OP.add
            )

            # circular conv: SH = s1*G + s0*roll(G,-1) + s2*roll(G,+1)
            SH = lp.tile([128, W], F32)
            nc.vector.tensor_scalar_mul(SH[:], G[:], F_s1)
            nc.vector.scalar_tensor_tensor(
                SH[:, 0:W - 1], G[:, 1:W], F_s0, SH[:, 0:W - 1],
                op0=OP.mult, op1=OP.add,
            )
            nc.vector.scalar_tensor_tensor(
                SH[:, 1:W], G[:, 0:W - 1], F_s2, SH[:, 1:W],
                op0=OP.mult, op1=OP.add,
            )
            # boundary columns: SH[p, W-1] += s0 * G[p+1 (wrap in batch), 0]
            # and SH[p, 0] += s2 * G[p-1 (wrap), W-1].  One bf16 matmul per
            # direction against a constant in-batch shift permutation does
            # the partition shift (out[m] = sum_p perm[p, m] * G[p, col]).
            Gb = lp.tile([128, 2], BF16)
            nc.vector.tensor_copy(Gb[:, 0:1], G[:, 0:1])
            nc.vector.tensor_copy(Gb[:, 1:2], G[:, W - 1:W])
            bl_ps = psB.tile([128, 1], F32, tag="tiny")
            nc.tensor.matmul(bl_ps[:], lhsT=shifts_bf[:, 0:128], rhs=Gb[:, 0:1],
                             start=True, stop=True)
            bl = lp.tile([128, 1], F32)
            nc.vector.tensor_copy(bl[:], bl_ps[:])
            nc.vector.scalar_tensor_tensor(
                SH[:, W - 1:W], bl[:], F_s0, SH[:, W - 1:W],
                op0=OP.mult, op1=OP.add,
            )
            br_ps = psB.tile([128, 1], F32, tag="tiny")
            nc.tensor.matmul(br_ps[:], lhsT=shifts_bf[:, 128:256], rhs=Gb[:, 1:2],
                             start=True, stop=True)
            br = lp.tile([128, 1], F32)
            nc.vector.tensor_copy(br[:], br_ps[:])
            nc.vector.scalar_tensor_tensor(
                SH[:, 0:1], br[:], F_s2, SH[:, 0:1], op0=OP.mult, op1=OP.add
            )

            # sharpening: P2 = (SH + 1e-8)^gamma = exp(gamma * ln(SH + 1e-8))
            Lg = lp.tile([128, W], F32)
            nc.scalar.activation(Lg[:], SH[:], AF.Ln, bias=eps_col)
            P2 = lp.tile([128, W], F32)
            rs2 = lp.tile([128, 1], F32)
            nc.scalar.activation(P2[:], Lg[:], AF.Exp, scale=F_gamma, accum_out=rs2[:])
            S2_ps = psB.tile([BL, 1], F32, tag="tiny")
            nc.tensor.matmul(S2_ps[:], lhsT=ind_sb, rhs=rs2[:], start=True, stop=True)
            S2 = lp.tile([BL, 1], F32)
            nc.vector.tensor_scalar_add(S2[:], S2_ps[:], EPS)
            r2 = lp.tile([BL, 1], F32)
            nc.vector.reciprocal(r2[:], S2[:])
            F3_ps = psB.tile([128, 1], F32, tag="tiny")
            nc.tensor.matmul(F3_ps[:], lhsT=indT_sb, rhs=r2[:], start=True, stop=True)
            F_r2 = lp.tile([128, 1], F32)
            nc.vector.tensor_copy(F_r2[:], F3_ps[:])

            P2sc = lp.tile([128, 1], F32)
            nc.vector.tensor_copy(P2sc[:], P2[:, 0:1])  # DVE observes ACT@P2
            outsb = lp.tile([128, W], F32)
            nc.vector.tensor_scalar_mul(outsb[:], P2[:], F_r2[:])
            nc.sync.dma_start(
                out_d[:].rearrange("b (q f) -> (b q) f", f=W), outsb[:]
            )
    nc.compile()
    return nc


def _get_nc():
    global _NC
    if _NC is None:
        _NC = build_nc()
    return _NC


def _enable_profiling():
    """Install the axon NTFF profile hook; the agent image lacks
    antenv.axon_hooks, so shim it and register the ctypes-based hook."""
    import types

    import concourse.bass_utils as bu

    bu.upload_artifacts = lambda tmpdir: tmpdir  # no artifact bucket here
    try:
        from antenv.axon_hooks import get_axon_ntff_profile_hook  # noqa: F401

        return
    except ImportError:
        pass
    import antenv

    mod = types.ModuleType("antenv.axon_hooks")
    _holder = {}
    mod.set_axon_ntff_profile_hook = lambda h: _holder.__setitem__("h", h)
    mod.get_axon_ntff_profile_hook = lambda: _holder.get("h")
    sys.modules["antenv.axon_hooks"] = mod
    antenv.axon_hooks = mod
    from trn_agent_boot.trn_boot import _ntff_profile_via_ctypes

    mod.set_axon_ntff_profile_hook(
        _ntff_profile_via_ctypes("/opt/axon/libaxon_pjrt.so")
    )


def kernel(**inputs):
    global LAST_RESULTS
    mem = np.ascontiguousarray(np.asarray(inputs["memory"], dtype=np.float32))
    cs = np.ascontiguousarray(np.asarray(inputs["controller_state"], dtype=np.float32))
    pw = np.ascontiguousarray(np.asarray(inputs["previous_weights"], dtype=np.float32))
    Wk = np.asarray(inputs["Wk"], np.float32)
    Wcat = np.concatenate(
        [
            np.asarray(inputs["Wb"], np.float32),
            np.asarray(inputs["Wg"], np.float32),
            np.asarray(inputs["Ws"], np.float32),
            np.asarray(inputs["Wgam"], np.float32),
        ],
        axis=1,
    )
    brow = np.concatenate(
        [
            np.asarray(inputs["bb"], np.float32),
            np.asarray(inputs["bg"], np.float32),
            np.asarray(inputs["bs"], np.float32),
            np.asarray(inputs["bgam"], np.float32),
        ]
    )

    # shard: core c gets batches [c*BL, (c+1)*BL); memory pre-transposed to (BL, D, N)
    memT = np.ascontiguousarray(
        mem.reshape(NCORES, BL, N, D).transpose(0, 1, 3, 2)
    )
    import ml_dtypes
    memT = memT.astype(ml_dtypes.bfloat16)
    csT = cs.reshape(NCORES, BL, C).transpose(0, 2, 1)  # (cores, C, BL)
    pw_sh = pw.reshape(NCORES, BL, N)

    smalls = np.zeros((NCORES, 128, SC), np.float32)
    smalls[:, :, S_CST:S_CST + BL] = csT[:, 0:128, :]
    smalls[:, :, S_CST + BL:S_CST + 2 * BL] = csT[:, 128:256, :]
    smalls[:, :, S_WK:S_WK + D] = Wk[0:128, :]
    smalls[:, :, S_WK + D:S_WK + 2 * D] = Wk[128:256, :]
    smalls[:, :, S_WC:S_WC + 6] = Wcat[0:128, :]
    smalls[:, :, S_WC + 6:S_WC + 12] = Wcat[128:256, :]
    smalls[:, 0:BL, S_B6:S_B6 + 6] = brow[None, None, :]

    in_maps = [
        {
            "memT": memT[c],
            "smalls": np.ascontiguousarray(smalls[c]),
            "pw": np.ascontiguousarray(pw_sh[c]),
        }
        for c in range(NCORES)
    ]
    nc = _get_nc()
    if PROFILE:
        _enable_profiling()
    res = run_bass_kernel_spmd(nc, in_maps, list(range(NCORES)), trace=PROFILE)
    LAST_RESULTS = res
    out = np.concatenate([r["out"] for r in res.results], axis=0)
    return out.astype(np.float32)
